# revision 36
# baseline (speedup 1.0000x reference)
"""Trainium2 Bass kernel for nn_Conv: per-token 16x8 image, 3x3 valid conv,
output flattened to first 84 of 128 slots, rest zero, ReLU.

Strategy (hardcoded for x:[256,1024,128] fp32, kernel:[3,3] fp32, 8 cores):
  - Pure data parallel: batch 256 -> 32 per core. Per-core tokens = 32*1024 = 32768.
  - conv == x[tok, 128] @ M[128, 84] with M built on host from the 3x3 kernel.
  - The host packs x to bf16 AND pre-transposes each 128-token block to
    [pixel, token] layout (one strided numpy copy, ~130ms for all cores), so
    the device runs ZERO PE transposes and ZERO PSUM->SBUF evacuation
    copies: per 128-token block one bf16 matmul (lhsT = xT block straight
    from the input DMA, rhs = M) into a bank-padded [P, 2, 512] fp32 PSUM
    tile (8 blocks per tile), then one ReLU moves 8x84 results to the
    output tile, alternating DVE/ACT so both engines share the work.
  - Blocks stay partition-major (block j = tokens {p*32 + j}) so the
    compact bf16 output [tokens, 84] DMAs out with contiguous 5.4KB
    partition rows; host pads the 44 zero columns + upcasts (outputs are
    donated zero buffers, so untouched pad columns read back as zero).
  - Input DMA chunks ride HWDGE (sync) lanes, outputs ride SWDGE (gpsimd)
    lanes, one small M DMA up front.  Every consumer waits on a single
    engine's semaphore where possible; _split_excess_waits NoOp-splits any
    extra waits (walrus allows one sync-wait per instruction).
"""

from contextlib import ExitStack

import numpy as np

import concourse.bass as bass
import concourse.tile as tile
from concourse import mybir
from concourse.bass_utils import run_bass_kernel_spmd

L, W, K = 16, 8, 3
B, S = 256, 1024
PX = L * W  # 128 pixels per token
OUT = (L - K + 1) * (W - K + 1)  # 84 conv outputs per token
N_CORES = 8
B_SHARD = B // N_CORES  # 32
TOKENS = B_SHARD * S  # 32768 tokens per core

CHUNK_TOKENS = 4096  # tokens per DMA chunk
T_PER_PART = CHUNK_TOKENS // 128  # 32 token-blocks per chunk
N_CHUNKS = TOKENS // CHUNK_TOKENS  # 8
P = 128
GC = 16  # token-blocks per relu group (4 PSUM banks of 4 x 84 fp32)
GR = 4  # token-blocks per PSUM bank (4 * 84 <= 512 fp32)
N_RGROUPS = T_PER_PART // GC  # 8 relu groups per chunk


def _build_conv_matrix(kernel3x3: np.ndarray) -> np.ndarray:
    """M[p, o]: coefficient of pixel p in conv output slot o."""
    m = np.zeros((PX, OUT), dtype=np.float32)
    oh, ow = L - K + 1, W - K + 1
    for oy in range(oh):
        for ox in range(ow):
            for ky in range(K):
                for kx in range(K):
                    m[(oy + ky) * W + (ox + kx), oy * ow + ox] += kernel3x3[ky, kx]
    return m


def _build_program():
    nc = bass.Bass(
        "TRN2", target_bir_lowering=False, debug=False, num_devices=N_CORES
    )
    f32 = mybir.dt.float32
    bf16 = mybir.dt.bfloat16
    # xt[c, px, t, p] = x[c*4096 + p*32 + t, px]: per (c, px) partition line
    # the 32x128 block-matrix is contiguous (8KB rows).
    xt_ap = nc.dram_tensor(
        "xt", [N_CHUNKS * P, T_PER_PART * P], bf16, kind="ExternalInput"
    ).ap()
    m_ap = nc.dram_tensor("m", [PX, OUT], bf16, kind="ExternalInput").ap()
    out_ap = nc.dram_tensor("out_c", [TOKENS, OUT], bf16, kind="ExternalOutput").ap()

    xtv = xt_ap.rearrange("(c p) f -> c p f", p=P)
    ov = out_ap.rearrange("(c p t) f -> c p t f", p=P, t=T_PER_PART)

    N_XBUF = 4
    N_PSBUF = 2
    with tile.TileContext(nc) as tc, ExitStack() as ctx:
        consts = ctx.enter_context(tc.tile_pool(name="consts", bufs=1))
        x_pool = ctx.enter_context(tc.tile_pool(name="x", bufs=1))
        o_pool = ctx.enter_context(tc.tile_pool(name="o", bufs=1))
        ps_pool = ctx.enter_context(tc.tile_pool(name="ps", bufs=1, space="PSUM"))

        # Few, large, manually-rotated tiles: every tile object costs
        # event-semaphore setup/teardown instructions on all engines, which
        # showed up as ~9us of end-of-program semaphore chains.
        x_tiles = [
            x_pool.tile([P, T_PER_PART, P], bf16, name=f"x{i}", tag=f"x{i}")
            for i in range(N_XBUF)
        ]
        # First input chunk as early as possible.
        nc.sync.dma_start(x_tiles[0][:].rearrange("p t q -> p (t q)"), xtv[0])

        # Conv matrix, pre-cast to bf16 on the host: one small DMA.
        m_bf = consts.tile([P, OUT], bf16)
        nc.sync.dma_start(m_bf[:], m_ap)

        # One output tile for all chunks (subtile deps keep chunks
        # independent), three bank-padded PSUM tiles rotated across groups.
        o_big = o_pool.tile([P, N_CHUNKS, T_PER_PART, OUT], bf16, name="obig")
        ps_os = [
            ps_pool.tile([P, 4, 512], f32, name=f"pso{i}", tag=f"pso{i}")
            for i in range(N_PSBUF)
        ]

        for c in range(N_CHUNKS):
            if c > 0:
                nc.sync.dma_start(
                    x_tiles[c % N_XBUF][:].rearrange("p t q -> p (t q)"), xtv[c]
                )
            x_tile = x_tiles[c % N_XBUF]
            o_tile = o_big[:, c]

            for h in range(N_RGROUPS):
                # 8 matmuls into a bank-padded [P, 2, 512] fp32 tile (each
                # 84-wide output stays inside one 512-fp32 bank), then one
                # ReLU evacuates all 672 values, alternating DVE/ACT.
                ps_o = ps_os[(c * N_RGROUPS + h) % N_PSBUF]
                for j in range(GC):
                    b, jj = divmod(j, GR)
                    nc.tensor.matmul(
                        ps_o[:, b, jj * OUT : (jj + 1) * OUT],
                        lhsT=x_tile[:, GC * h + j, :],
                        rhs=m_bf[:],
                        start=True,
                        stop=True,
                    )
                dst = o_tile[:, GC * h : GC * (h + 1), :]
                src = ps_o[:, :, : GR * OUT]
                if h % 2 == 0:
                    nc.scalar.activation(
                        dst, src, mybir.ActivationFunctionType.Relu
                    )
                else:
                    nc.vector.tensor_scalar_max(dst, src, 0.0)
            # Compact bf16 outputs on SWDGE (gpsimd) lanes.
            nc.gpsimd.dma_start(ov[c], o_tile)

    _split_excess_waits(nc)
    return nc


_SKIP_TYPES = ("Branch", "SemWait")


def _split_excess_waits(nc):
    """Move all but one sync wait onto injected same-engine NoOps.

    Walrus allows a single sync-wait slot per compute/DMA instruction, but
    the tile scheduler can emit several (data deps + its event-accel /
    bank-safety pacing waits).  A NoOp on the same engine immediately before
    the instruction stalls the queue identically, so semantics (including
    the pacing the hardware workarounds rely on) are preserved exactly.
    """
    counter = [0]
    for f in nc.m.functions:
        for blk in f.blocks:
            insts = blk.instructions
            i = 0
            while i < len(insts):
                inst = insts[i]
                si = inst.sync_info
                tname = type(inst).__name__
                if (
                    si is not None
                    and len(si.on_wait) > 1
                    and not any(s in tname for s in _SKIP_TYPES)
                ):
                    waits = list(si.on_wait)
                    for w in waits[:-1]:
                        counter[0] += 1
                        nop = mybir.InstNoOp(
                            name=f"wsplit-{counter[0]}", ins=[], outs=[]
                        )
                        nop.engine = inst.engine
                        nop.sync_info = mybir.SyncInfo(on_wait=[w], on_update=[])
                        insts.insert(i, nop)
                        i += 1
                    inst.sync_info = mybir.SyncInfo(
                        on_wait=[waits[-1]], on_update=list(si.on_update)
                    )
                i += 1


_PROGRAM_CACHE = {}


def _get_program():
    if "nc" not in _PROGRAM_CACHE:
        _PROGRAM_CACHE["nc"] = _build_program()
    return _PROGRAM_CACHE["nc"]


def _make_in_maps(x: np.ndarray, m: np.ndarray) -> list:
    import ml_dtypes

    xb = np.ascontiguousarray(x).reshape(B, S * PX).astype(ml_dtypes.bfloat16)
    maps = []
    for i in range(N_CORES):
        shard = xb[i * B_SHARD : (i + 1) * B_SHARD].reshape(TOKENS, PX)
        # xt[c, px, t, p] = shard[c*4096 + p*32 + t, px]
        xt = np.ascontiguousarray(
            shard.reshape(N_CHUNKS, P, T_PER_PART, PX).transpose(0, 3, 2, 1)
        ).reshape(N_CHUNKS * P, T_PER_PART * P)
        maps.append({"xt": xt, "m": m.astype(ml_dtypes.bfloat16)})
    return maps


def kernel(x: np.ndarray, kernel: np.ndarray) -> np.ndarray:
    x = np.ascontiguousarray(np.asarray(x, dtype=np.float32))
    k3 = np.asarray(kernel, dtype=np.float32)
    assert x.shape == (B, S, PX), x.shape
    assert k3.shape == (K, K), k3.shape

    m = _build_conv_matrix(k3)

    nc = _get_program()
    res = run_bass_kernel_spmd(nc, _make_in_maps(x, m), list(range(N_CORES)))
    out = np.zeros((B, S, PX), dtype=np.float32)
    for i in range(N_CORES):
        out[i * B_SHARD : (i + 1) * B_SHARD, :, :OUT] = (
            res.results[i]["out_c"].astype(np.float32).reshape(B_SHARD, S, OUT)
        )
    return out
